# revision 13
# baseline (speedup 1.0000x reference)
"""CrossAttnBlock TRN2 kernel: 8-way (batch x l-half) sharded, collective-free.

Reference math (b=4, c=64, h=64, w=32, dim=256, HEADS=8, l=h*w=2048):
  zf = z.reshape(b, dim, l).T            # [b, l, dim]
  q  = x.reshape(b, c, l).T              # [b, l, c]
  k  = (zf @ Wk + bk) -> [b, H, l, c];  v likewise
  S  = q @ k.T / sqrt(c); A = softmax(S, -1); P = A @ v
  out = (P heads-concat) @ Wo + bo       # [b, l, c]
  return x + out.reshape(b, c, h, w)     # raw-memory reinterpretation

Per-core (core = bi*2 + half): full K/V projection for batch bi, attention +
out-proj for l rows [half*1024, (half+1)*1024).

v2 optimizations over the first working version:
 - bk dropped entirely (a per-row additive constant cancels in softmax);
   bv/bo folded on the host into the residual input (softmax rows sum to 1,
   so A @ (V + 1 bv^T) @ Wo + bo = A@V@Wo + (bv@Wo + bo)).
 - Score matmuls for the two heads sharing a kT tile are interleaved so the
   row-group-0 (partitions 0-63) and row-group-64 matmuls run concurrently
   on the PE array (contraction is only c=64).
 - The softmax exp is split between ScalarE (exact table exp) and VectorE
   (Schraudolph bit-trick exp: es_bf16_bits = int16(S*A + B), exploiting the
   f32->bf16 bit layout; softmax self-normalization cancels the ~3% element
   error to ~1e-5 at the output).
 - Phase D normalize+accumulate fused into one scalar_tensor_tensor per head.
 - Projection PSUM->SBUF copies merged into [128,1024] ops and spread across
   VectorE and ScalarE; input DMAs spread across engine queues.
"""
import ml_dtypes
import numpy as np

import concourse.bass as bass
import concourse.mybir as mybir
import concourse.tile as tile
from concourse import bacc
from concourse.bass_utils import run_bass_kernel_spmd
from concourse.masks import make_identity

F32 = mybir.dt.float32
BF16 = mybir.dt.bfloat16
I8 = mybir.dt.int8
F8 = mybir.dt.float8e4

B, C, H, W = 4, 64, 64, 32
DIM = 256
HEADS = 8
L = H * W            # 2048
LH = L // 2          # 1024 per core
INNER = HEADS * C    # 512
N_CORES = 8

# Schraudolph exp constants (exp(x/8) via int8 bitcast to fp8e4m3), tuned for
# min worst-case relative error (~7%) incl. trunc-toward-zero compensation;
# softmax self-normalization cancels this to ~1e-4 at the output.
EXP_A8 = 1.4426950408889634
EXP_B8 = 56.132

_CACHE = {}


def _dve_exp(h, mt):
    # ~43 of 128 exp tiles on VectorE, rest on ScalarE (load balance)
    return (h + mt) % 3 == 1


def build_nc():
    nc = bacc.Bacc("TRN2", target_bir_lowering=False, debug=False,
                   num_devices=N_CORES)
    xq = nc.dram_tensor("xq", [C, LH], BF16, kind="ExternalInput")
    xr = nc.dram_tensor("xr", [128, LH // 128, C], F32, kind="ExternalInput")
    zb = nc.dram_tensor("zb", [DIM, L], BF16, kind="ExternalInput")
    Wk = nc.dram_tensor("Wk", [DIM, INNER], BF16, kind="ExternalInput")
    Wv = nc.dram_tensor("Wv", [DIM, INNER], BF16, kind="ExternalInput")
    Wo = nc.dram_tensor("Wo", [C, HEADS, C], BF16, kind="ExternalInput")
    OUT = nc.dram_tensor("out", [LH, C], F32, kind="ExternalOutput")

    NMT = L // 128       # 16 m-tiles
    NLS = LH // 128      # 8 l-subtiles

    with tile.TileContext(nc) as tc:
        with (
            tc.tile_pool(name="const", bufs=1) as cp,
            tc.tile_pool(name="pexp", bufs=2) as pe_pool,
            tc.tile_pool(name="small", bufs=3) as sp,
            tc.tile_pool(name="ps", bufs=1, space="PSUM") as ps,
        ):
            # ---- constants / inputs in SBUF (DMAs spread across queues) ----
            z_sb = [cp.tile([128, L], BF16, tag=f"z{d}", name=f"z{d}") for d in range(2)]
            wk_sb = [cp.tile([128, INNER], BF16, tag=f"wk{d}", name=f"wk{d}") for d in range(2)]
            wv_sb = [cp.tile([128, INNER], BF16, tag=f"wv{d}", name=f"wv{d}") for d in range(2)]
            nc.sync.dma_start(out=wk_sb[0], in_=Wk[0:128, :])
            nc.scalar.dma_start(out=wk_sb[1], in_=Wk[128:256, :])
            # chunked so Phase A's first matmuls start after ~512KB, not 1MB
            nc.sync.dma_start(out=z_sb[0][:, 0:1024], in_=zb[0:128, 0:1024])
            nc.scalar.dma_start(out=z_sb[1][:, 0:1024], in_=zb[128:256, 0:1024])
            nc.sync.dma_start(out=z_sb[0][:, 1024:2048], in_=zb[0:128, 1024:2048])
            nc.scalar.dma_start(out=z_sb[1][:, 1024:2048], in_=zb[128:256, 1024:2048])
            nc.sync.dma_start(out=wv_sb[0], in_=Wv[0:128, :])
            nc.scalar.dma_start(out=wv_sb[1], in_=Wv[128:256, :])
            wo_sb = cp.tile([C, HEADS, C], BF16, tag="wo")
            nc.gpsimd.dma_start(out=wo_sb, in_=Wo[:, :, :])
            x_sb = cp.tile([128, LH], BF16, tag="x")
            nc.gpsimd.dma_start(out=x_sb[0:C, :], in_=xq[:, :])
            nc.gpsimd.dma_start(out=x_sb[C:2 * C, :], in_=xq[:, :])
            xr_sb = cp.tile([128, NLS, C], F32, tag="xr")
            nc.gpsimd.dma_start(out=xr_sb, in_=xr[:, :, :])
            ident = cp.tile([8, 8], BF16, tag="ident")
            make_identity(nc, ident)

            kT_sb = [cp.tile([128, L], BF16, tag=f"kT{t}", name=f"kT{t}") for t in range(4)]
            v_sb = cp.tile([128, NMT, HEADS, C + 2], F8, tag="v")
            nc.vector.memset(v_sb[:, :, :, C:C + 1], 1.0)
            pt_sb = [cp.tile([C + 1, LH], BF16, tag=f"pt{h}", name=f"pt{h}") for h in range(HEADS)]
            sums_sb = cp.tile([HEADS, LH], BF16, tag="sums")

            # ---- Phase A: kT[ci, m] = Wk^T @ zf^T ----
            for s2 in range(2):           # consume z in DMA-chunk order
                for t in range(4):
                    pk = ps.tile([128, 1024], F32, tag="s2", bufs=2, name="pk")
                    for half in range(2):
                        for d in range(2):
                            nc.tensor.matmul(
                                pk[:, half * 512:(half + 1) * 512],
                                wk_sb[d][:, t * 128:(t + 1) * 128],
                                z_sb[d][:, s2 * 1024 + half * 512:s2 * 1024 + (half + 1) * 512],
                                start=(d == 0), stop=(d == 1))
                    dst = kT_sb[t][:, s2 * 1024:(s2 + 1) * 1024]
                    if t % 2 == 0:
                        nc.vector.tensor_copy(out=dst, in_=pk)
                    else:
                        nc.scalar.copy(out=dst, in_=pk)
            # ---- Phase C: attention. Scores rotate through four single-bank
            # [128,512] PSUM chunks (bank-granular WAR gives one full mt of
            # pipeline slack), the head pair's score matmuls issue adjacently
            # and run concurrently on PE row groups 0-63 / 64-127. exp is
            # chunk-granular: head0 on ScalarE (table exp), head1 on VectorE
            # (Schraudolph int16 bit-trick). ----
            for t in range(4):
                h0, h1 = 2 * t, 2 * t + 1
                ptp = [ps.tile([C + 1, LH], F32, tag=f"pt{i}", name=f"ptp{i}")
                       for i in range(2)]
                # software-pipelined: the DoubleRow AV for m-tile pair mp is
                # emitted after S/exp(mp+1) so the in-order PE queue never
                # parks on an exp wait.
                es_pair = {}
                for mt in range(NMT + 2):
                    if t == 0 and mt < NMT:
                        # Phase B woven in: v-projection for this m-tile,
                        # ready one step before AV(pair0, mt) consumes it
                        pv = ps.tile([128, 512], F32, tag="s2", bufs=2, name="pv")
                        for d in range(2):
                            nc.tensor.matmul(
                                pv,
                                z_sb[d][:, mt * 128:(mt + 1) * 128],
                                wv_sb[d],
                                start=(d == 0), stop=(d == 1))
                        dstv = v_sb[:, mt, :, 0:C]
                        srcv = pv.rearrange("p (h c) -> p h c", h=HEADS)
                        if mt % 2 == 0:
                            nc.vector.tensor_copy(out=dstv, in_=srcv)
                        else:
                            nc.scalar.copy(out=dstv, in_=srcv)
                    if mt < NMT:
                        if mt % 2 == 0:
                            es_pair[mt // 2] = [
                                pe_pool.tile([128, 2, LH], F8, tag=tg, bufs=3,
                                             name=f"es_{tg}")
                                for tg in ("esA", "esB")]
                        es = es_pair[mt // 2]
                        for lh_ in range(2):
                            pch = ps.tile([128, 1024], F32, tag="s2", bufs=2, name="pch")
                            for i, roff in ((0, 0), (1, 64)):
                                nc.tensor.matmul(
                                    pch[:, i * 512:(i + 1) * 512],
                                    kT_sb[t][roff:roff + 64, mt * 128:(mt + 1) * 128],
                                    x_sb[roff:roff + C, lh_ * 512:(lh_ + 1) * 512],
                                    start=True, stop=True)
                            nc.scalar.activation(
                                out=es[0][:, mt % 2, lh_ * 512:(lh_ + 1) * 512],
                                in_=pch[:, 0:512],
                                func=mybir.ActivationFunctionType.Exp,
                                scale=float(C) ** -0.5)
                            nc.vector.tensor_scalar(
                                out=es[1].bitcast(I8)[:, mt % 2, lh_ * 512:(lh_ + 1) * 512],
                                in0=pch[:, 512:1024],
                                scalar1=EXP_A8, scalar2=EXP_B8,
                                op0=mybir.AluOpType.mult,
                                op1=mybir.AluOpType.add)
                    if mt >= 2 and mt % 2 == 0:
                        mp = (mt - 2) // 2
                        esp = es_pair.pop(mp)
                        for i, h in ((0, h0), (1, h1)):
                            for lh_ in range(2):
                                nc.tensor.matmul(
                                    ptp[i][:, lh_ * 512:(lh_ + 1) * 512],
                                    v_sb[:, 2 * mp:2 * mp + 2, h, 0:C + 1],
                                    esp[i][:, :, lh_ * 512:(lh_ + 1) * 512],
                                    start=(mp == 0), stop=(mp == NMT // 2 - 1),
                                    perf_mode=mybir.MatmulPerfMode.DoubleRow)
                for i, h in ((0, h0), (1, h1)):
                    nc.scalar.copy(out=pt_sb[h][:, 0:512],
                                   in_=ptp[i][0:C + 1, 0:512])
                    nc.vector.tensor_copy(out=pt_sb[h][:, 512:1024],
                                          in_=ptp[i][0:C + 1, 512:1024])
                    nc.sync.dma_start(out=sums_sb[h:h + 1, :],
                                      in_=pt_sb[h][C:C + 1, :])

            # ---- Phase D: out-proj + normalize + residual per l-subtile.
            # Even heads normalize+accumulate on VectorE (fused STT); odd
            # heads normalize on ScalarE into bf16 staging, added by VectorE
            # at 2x rate, halving the VectorE queue time of this tail. ----
            for ls in range(NLS):
                ptr = ps.tile([128, 8], BF16, tag="s2", bufs=2, name="ptr")
                nc.tensor.transpose(ptr, sums_sb[:, ls * 128:(ls + 1) * 128], ident)
                recip = sp.tile([128, 8], F32, tag="recip")
                nc.vector.reciprocal(out=recip, in_=ptr)
                acc = sp.tile([128, C], F32, tag="oacc")
                tmp = [sp.tile([128, C], BF16, tag=f"tmp{j}", name=f"tmp{j}")
                       for j in range(4)]
                for h in range(HEADS):
                    po = ps.tile([128, C], F32, tag=("pt0" if h % 2 == 0 else "pt1"), name="po")
                    nc.tensor.matmul(
                        po,
                        pt_sb[h][0:C, ls * 128:(ls + 1) * 128],
                        wo_sb[:, h, :],
                        start=True, stop=True)
                    if h % 2 == 0:
                        nc.vector.scalar_tensor_tensor(
                            out=acc, in0=po, scalar=recip[:, h:h + 1],
                            in1=(xr_sb[:, ls, :] if h == 0 else acc),
                            op0=mybir.AluOpType.mult,
                            op1=mybir.AluOpType.add)
                    else:
                        nc.scalar.activation(
                            out=tmp[h // 2], in_=po,
                            func=mybir.ActivationFunctionType.Copy,
                            scale=recip[:, h:h + 1])
                for j in range(4):
                    nc.vector.tensor_tensor(
                        out=acc, in0=acc, in1=tmp[j],
                        op=mybir.AluOpType.add)
                nc.sync.dma_start(out=OUT[ls * 128:(ls + 1) * 128, :], in_=acc)

    nc.compile()
    return nc


def kernel(x, z, Wk, bk, Wv, bv, Wo, bo):
    x = np.ascontiguousarray(x, dtype=np.float32)
    z = np.ascontiguousarray(z, dtype=np.float32)
    if "nc" not in _CACHE:
        _CACHE["nc"] = build_nc()
    nc = _CACHE["nc"]
    # bv/bo fold into a constant output row (softmax rows sum to 1); bk
    # cancels inside the softmax.
    bconst = (np.asarray(bv, np.float32) @ np.asarray(Wo, np.float32)
              + np.asarray(bo, np.float32))           # [C]
    shared = {
        "Wk": np.ascontiguousarray(np.asarray(Wk, np.float32).astype(ml_dtypes.bfloat16)),
        "Wv": np.ascontiguousarray(np.asarray(Wv, np.float32).astype(ml_dtypes.bfloat16)),
        "Wo": np.ascontiguousarray(np.asarray(Wo, np.float32)
                                   .reshape(HEADS, C, C).transpose(1, 0, 2)
                                   .astype(ml_dtypes.bfloat16)),
    }
    in_maps = []
    for core in range(N_CORES):
        bi, half = core // 2, core % 2
        xi = x[bi].reshape(C, L)
        in_maps.append({
            "xq": np.ascontiguousarray(
                xi[:, half * LH:(half + 1) * LH].astype(ml_dtypes.bfloat16)),
            "xr": np.ascontiguousarray(
                (x[bi].reshape(-1)[half * LH * C:(half + 1) * LH * C]
                 .reshape(LH // 128, 128, C) + bconst)
                .transpose(1, 0, 2)),
            "zb": np.ascontiguousarray(
                z[bi].reshape(DIM, L).astype(ml_dtypes.bfloat16)),
            **shared,
        })
    _CACHE["in_maps"] = in_maps
    res = run_bass_kernel_spmd(nc, in_maps, list(range(N_CORES)))
    full = np.empty((B, L * C), dtype=np.float32)
    for core in range(N_CORES):
        bi, half = core // 2, core % 2
        full[bi, half * LH * C:(half + 1) * LH * C] = \
            res.results[core]["out"].reshape(-1)
    return full.reshape(B, C, H, W)
